# revision 21
# baseline (speedup 1.0000x reference)
"""Trainium2 Bass kernel for batched GNN message passing.

Computes, for x:[L,G,D], COO edges (rows, cols, vals), W:[D,D], b, gamma, beta:
    xt  = x.transpose(1,0,2).reshape(G, L*D)
    agg = segment_sum(xt[cols] * vals[:,None], rows, G)     # [G, L*D]
    h   = einsum('lgd,od->lgo', agg_as_lgd, W) + b
    s   = silu(h)
    out = layernorm(s) * gamma + beta                        # LN over D

Strategy (v3): destination rows are LPT-packed on the host into 392
balanced 128-row blocks (49 per core, 16 tiles of 128 edges each).  The
host routes each edge to its destination block and lays the source
features xt[cols] out as dense bf16 message tiles M[block][p][tile][L*D]
(a pure permutation/copy), streamed to the device with large regular
HWDGE DMAs — no gpsimd dma_gather (whose per-index SWDGE descriptor
generation was the v1 bottleneck), and no gpsimd compute at all (its
SBUF port sharing with the vector engine poisons DVE throughput).

The per-tile one-hot-times-val selection matrix S[e,r] = val_e*(rowloc_e
== r) for ALL 16 tiles of a block is built in ONE custom-DVE instruction:
host packs enc[p,t] = rowloc + val (val in (0,1); val==0 edges dropped),
and the op computes t = enc - iota_r; S = relu(t)*(t <= 1), which equals
val exactly at r == rowloc and 0 elsewhere.  The segment-sum runs
directly in transposed form on the PE: aggT[d,r] += M_l[e,d].T @ S[e,r],
so the 128x128 linear consumes aggT with no on-chip transpose.  SiLU and
the squared-sum for LayerNorm run on ACT with stream accumulators.
"""

import numpy as np

L, G, D, E = 2, 50000, 128, 800000
N_CORES = 8
P = 128
NBLK = 49                     # block slots per core
NBLK_TOT = N_CORES * NBLK     # 392 blocks of 128 rows = 50176 slots
RPC = NBLK * P                # padded rows per core = 6272
F = L * D                     # 256 packed feature width
LN_EPS = 1e-5

_CACHE: dict = {}
_GNN_SEL = None


def _register_dve_op():
    """Register (once) the custom DVE op building val*onehot(rowloc) tiles.

    out[p, s, k] = relu(t) * (t <= 1),  t = in1[p, s, 0] - in0[p, 0, k]
    With in0 = iota (k) and in1 = rowloc + val (val in (0,1]):
      k == rowloc -> t = val  -> out = val
      k <  rowloc -> t >= 1+val > 1 -> masked to 0 (val > 0)
      k >  rowloc -> t <= val-1 <= 0 -> relu gives 0
      padding (enc = 0) -> t = -k <= 0 -> 0
    """
    global _GNN_SEL
    if _GNN_SEL is not None:
        return _GNN_SEL
    import re

    from concourse import dve_ops
    from concourse.dve_spec import One, Spec, Src0, Src1, relu

    for op in dve_ops.OPS:
        if op.name == "GNN_ONEHOT_SEL":
            _GNN_SEL = op
            return op

    t = Src1 - Src0
    body = relu(t) * (t <= One)
    spec = Spec(
        body=body,
        reference=lambda in0, in1, *a: np.where(
            (in1 - in0 > 0) & (in1 - in0 <= 1), in1 - in0, 0.0
        ).astype(np.float32),
    )
    op = dve_ops.DveOp("GNN_ONEHOT_SEL", spec, subdim=False, uops_sha={})
    dve_ops.OPS.append(op)
    row = dve_ops._CUSTOM_DVE_ROW_BASE + len(dve_ops.OPS) - 1
    assert row < 0x20, "custom-DVE row field overflow"
    dve_ops._SUB_OPCODE_FOR_NAME[op.name] = row
    dve_ops.CUSTOM_DVE_SPECS[op.name] = spec
    for ver in ("v3", "v4"):
        try:
            op.compile(ver)
        except ValueError as e:
            m = re.search(r'uops_sha\["%s"\]="([0-9a-f]+)"' % ver, str(e))
            if m:
                op.uops_sha[ver] = m.group(1)
        try:
            op.compile(ver)
        except ValueError:
            pass
    _GNN_SEL = op
    return op


def _build_program(TT, apply_bias, apply_gamma, apply_beta):
    import concourse.bacc as bacc
    import concourse.mybir as mybir
    import concourse.tile as tile

    sel_op = _register_dve_op()

    f32 = mybir.dt.float32
    bf16 = mybir.dt.bfloat16
    Alu = mybir.AluOpType
    Act = mybir.ActivationFunctionType

    TTmax = max(TT)
    NCOL = NBLK * L

    nc = bacc.Bacc(None, target_bir_lowering=False, debug=False)

    m_d = nc.dram_tensor("m", [NBLK, P, TTmax * F], bf16, kind="ExternalInput")
    enc_d = nc.dram_tensor("enc", [P, NBLK * TTmax], f32, kind="ExternalInput")
    wt_d = nc.dram_tensor("wt", [P, P], bf16, kind="ExternalInput")
    iota_d = nc.dram_tensor("iota", [P, P], f32, kind="ExternalInput")
    if apply_bias:
        bias_d = nc.dram_tensor("bias", [P, P], f32, kind="ExternalInput")
    if apply_gamma:
        gamma_d = nc.dram_tensor("gamma", [P, P], f32, kind="ExternalInput")
    if apply_beta:
        beta_d = nc.dram_tensor("beta", [P, P], f32, kind="ExternalInput")
    out_d = nc.dram_tensor("out", [RPC, L, D], f32, kind="ExternalOutput")

    with tile.TileContext(nc) as tc:
        with (
            tc.tile_pool(name="const", bufs=1) as constp,
            tc.tile_pool(name="mbuf", bufs=4) as mpool,
            tc.tile_pool(name="sbuild", bufs=4) as spool,
            tc.tile_pool(name="mid", bufs=6) as midpool,
            tc.tile_pool(name="store", bufs=1) as store,
            tc.tile_pool(name="outp", bufs=4) as outp,
            tc.tile_pool(name="psA", bufs=2, space="PSUM") as psA,
            tc.tile_pool(name="psB", bufs=2, space="PSUM") as psB,
            tc.tile_pool(name="psH", bufs=2, space="PSUM") as psH,
        ):
            # enc for the first few blocks lands first so block 0's S-build
            # starts as early as possible; the bulk follows.
            iota_s = constp.tile([P, P], f32)
            nc.sync.dma_start(iota_s[:], iota_d[:])
            enc_s = constp.tile([P, NBLK * TTmax], f32)
            nc.sync.dma_start(enc_s[:, : 4 * TTmax], enc_d[:, : 4 * TTmax])
            wt_s = constp.tile([P, P], bf16)
            nc.sync.dma_start(wt_s[:], wt_d[:])
            nc.sync.dma_start(enc_s[:, 4 * TTmax :], enc_d[:, 4 * TTmax :])
            if apply_bias:
                bias_s = constp.tile([P, P], f32)
                nc.sync.dma_start(bias_s[:], bias_d[:])
            if apply_gamma:
                gamma_s = constp.tile([P, P], f32)
                nc.sync.dma_start(gamma_s[:], gamma_d[:])
            if apply_beta:
                beta_s = constp.tile([P, P], f32)
                nc.sync.dma_start(beta_s[:], beta_d[:])

            s_store = store.tile([P, NCOL * P], bf16)
            muvar = store.tile([P, NCOL, 2], f32)
            eps_t = store.tile([P, 1], f32)
            nc.vector.memset(eps_t[:], LN_EPS)

            # ---- Phase 1: stream M, build S, segment-sum, linear, SiLU ----
            for bi in range(NBLK):
                tt = TT[bi]

                M = mpool.tile([P, TTmax, F], bf16, tag="m")
                nc.sync.dma_start(M[:, :tt, :], m_d[bi][:, : tt * F])

                S_all = spool.tile([P, TTmax, P], bf16, tag="s")
                nc.vector._custom_dve(
                    sel_op,
                    out=S_all[:, :tt, :],
                    in0=iota_s[:].unsqueeze(1).broadcast_to([P, tt, P]),
                    in1=enc_s[:, bi * TTmax : bi * TTmax + tt]
                    .unsqueeze(2)
                    .broadcast_to([P, tt, P]),
                )

                # two accumulation chains in separate PSUM banks (interleaved
                # groups within one bank corrupt each other — HW-probed)
                agg0 = psA.tile([P, P], f32, tag="a0")
                agg1 = psB.tile([P, P], f32, tag="a1")
                for t in range(tt):
                    nc.tensor.matmul(
                        agg0[:], lhsT=M[:, t, 0:P], rhs=S_all[:, t, :],
                        start=(t == 0), stop=(t == tt - 1),
                    )
                for t in range(tt):
                    nc.tensor.matmul(
                        agg1[:], lhsT=M[:, t, P:F], rhs=S_all[:, t, :],
                        start=(t == 0), stop=(t == tt - 1),
                    )

                aT = midpool.tile([P, L, P], bf16, tag="aT")
                nc.scalar.activation(out=aT[:, 0, :], in_=agg0[:], func=Act.Copy)
                nc.scalar.activation(out=aT[:, 1, :], in_=agg1[:], func=Act.Copy)
                for l in range(L):
                    col = bi * L + l
                    h_ps = psH.tile([P, P], f32, tag="h")
                    nc.tensor.matmul(
                        h_ps[:], lhsT=aT[:, l, :], rhs=wt_s[:], start=True, stop=True
                    )
                    if apply_bias:
                        hb = outp.tile([P, P], f32, tag="hb")
                        nc.vector.tensor_tensor(
                            out=hb[:], in0=h_ps[:], in1=bias_s[:], op=Alu.add
                        )
                        silu_in = hb[:]
                    else:
                        silu_in = h_ps[:]
                    s_sl = s_store[:, col * P : (col + 1) * P]
                    nc.scalar.activation(out=s_sl, in_=silu_in, func=Act.Silu)
                    bn6 = outp.tile([P, 6], f32, tag="bn6")
                    nc.vector.bn_stats(bn6[:], s_sl)
                    nc.vector.bn_aggr(muvar[:, col, :], bn6[:])

                # fused LayerNorm + store for this block
                std2 = outp.tile([P, L], f32, tag="std2")
                nc.scalar.activation(
                    out=std2[:],
                    in_=muvar[:, bi * L : (bi + 1) * L, 1],
                    func=Act.Sqrt,
                    bias=eps_t[:],
                )
                rstd2 = outp.tile([P, L], f32, tag="rstd2")
                nc.vector.reciprocal(rstd2[:], std2[:])
                nmr2 = outp.tile([P, L], f32, tag="nmr2")
                nc.vector.tensor_tensor(
                    out=nmr2[:], in0=muvar[:, bi * L : (bi + 1) * L, 0],
                    in1=rstd2[:], op=Alu.mult,
                )
                nc.vector.tensor_scalar(
                    out=nmr2[:], in0=nmr2[:], scalar1=-1.0, scalar2=None,
                    op0=Alu.mult,
                )
                o_t = outp.tile([P, L, P], f32, tag="o")
                for l in range(L):
                    col = bi * L + l
                    nc.scalar.activation(
                        out=o_t[:, l, :],
                        in_=s_store[:, col * P : (col + 1) * P],
                        func=Act.Identity,
                        scale=rstd2[:, l : l + 1],
                        bias=nmr2[:, l : l + 1],
                    )
                    if apply_gamma:
                        nc.vector.tensor_tensor(
                            out=o_t[:, l, :], in0=o_t[:, l, :], in1=gamma_s[:],
                            op=Alu.mult,
                        )
                    if apply_beta:
                        nc.vector.tensor_tensor(
                            out=o_t[:, l, :], in0=o_t[:, l, :], in1=beta_s[:],
                            op=Alu.add,
                        )
                nc.sync.dma_start(out_d[bi * P : (bi + 1) * P], o_t[:])

    nc.compile()
    return nc


def _pack_rows(deg):
    """LPT-pack G rows into NBLK_TOT blocks of exactly P rows, balancing
    total edge load.  Returns (block_of_row, localrow_of_row, load)."""
    import heapq

    order = np.argsort(-deg, kind="stable")
    heap = [(0, 0, b) for b in range(NBLK_TOT)]  # (load, nrows, block)
    heapq.heapify(heap)
    block_of_row = np.empty(G, dtype=np.int64)
    localrow = np.empty(G, dtype=np.int64)
    load_arr = np.zeros(NBLK_TOT, dtype=np.int64)
    for r in order:
        while True:
            load, cnt, b = heapq.heappop(heap)
            if cnt < P:
                break
        block_of_row[r] = b
        localrow[r] = cnt
        load_arr[b] = load + deg[r]
        heapq.heappush(heap, (load + int(deg[r]), cnt + 1, b))
    return block_of_row, localrow, load_arr


def kernel(x, rows, cols, vals, W, b, gamma, beta):
    import ml_dtypes
    from concourse import bass_utils

    x = np.asarray(x, dtype=np.float32)
    rows = np.asarray(rows, dtype=np.int64)
    cols = np.asarray(cols, dtype=np.int64)
    vals = np.asarray(vals, dtype=np.float32)
    W = np.asarray(W, dtype=np.float32)
    b = np.asarray(b, dtype=np.float32)
    gamma = np.asarray(gamma, dtype=np.float32)
    beta = np.asarray(beta, dtype=np.float32)
    bf = ml_dtypes.bfloat16

    # zero-valued edges contribute nothing; drop them (required by the
    # enc = rowloc + val encoding, which needs val > 0)
    keep = vals != 0.0
    if not keep.all():
        rows, cols, vals = rows[keep], cols[keep], vals[keep]
    ne = len(rows)

    # ---- host-side routing: balanced destination blocks ----
    deg = np.bincount(rows, minlength=G)
    block_of_row, localrow, load = _pack_rows(deg)

    rank = np.argsort(-load, kind="stable")
    coremap = np.empty(NBLK_TOT, dtype=np.int64)
    slotmap = np.empty(NBLK_TOT, dtype=np.int64)
    for i in range(NBLK_TOT):
        coremap[rank[i]] = i % N_CORES
        slotmap[rank[i]] = i // N_CORES
    slot_load = np.zeros(NBLK, dtype=np.int64)
    for bk in range(NBLK_TOT):
        slot_load[slotmap[bk]] = max(slot_load[slotmap[bk]], load[bk])
    TT = [max(1, int(v)) for v in np.ceil(slot_load / P).astype(np.int64)]
    TTmax = max(TT)

    # ---- route edges ----
    eb = block_of_row[rows]
    core_e = coremap[eb]
    slot_e = slotmap[eb]
    rowloc_e = localrow[rows].astype(np.float32)
    gid = core_e * NBLK + slot_e
    order = np.argsort(gid, kind="stable")
    gid_s = gid[order]
    counts = np.bincount(gid_s, minlength=N_CORES * NBLK)
    starts = np.zeros(N_CORES * NBLK, dtype=np.int64)
    np.cumsum(counts[:-1], out=starts[1:])
    pos = np.arange(ne, dtype=np.int64) - starts[gid_s]
    t_arr = pos // P
    p_arr = pos % P
    core_s = core_e[order]
    slot_s = slot_e[order]

    # ---- message tiles: pure gather/permutation of xt, in bf16 ----
    xt = np.ascontiguousarray(x.transpose(1, 0, 2).reshape(G, F)).astype(bf)
    M_host = np.zeros((N_CORES, NBLK, P, TTmax, F), dtype=bf)
    M_host[core_s, slot_s, p_arr, t_arr] = xt[cols[order]]

    # enc[p, slot*TTmax + t] = rowloc + val   (0 in padding slots).
    # If val is so small that rowloc+val rounds to exactly rowloc, the
    # device decode would read it as a full-weight edge into rowloc-1;
    # zero it instead (its true contribution is < 8e-6).
    encv = rowloc_e[order] + vals[order]
    encv[encv == rowloc_e[order]] = 0.0
    enc = np.zeros((N_CORES, P, NBLK * TTmax), dtype=np.float32)
    enc[core_s, p_arr, slot_s * TTmax + t_arr] = encv

    wt = np.ascontiguousarray(W.T).astype(bf)
    iota_b = np.ascontiguousarray(
        np.tile(np.arange(P, dtype=np.float32), (P, 1))
    )

    apply_bias = bool(np.any(b != 0))
    apply_gamma = bool(np.any(gamma != 1))
    apply_beta = bool(np.any(beta != 0))

    key_prog = (tuple(TT), apply_bias, apply_gamma, apply_beta)
    if key_prog not in _CACHE:
        _CACHE[key_prog] = _build_program(TT, apply_bias, apply_gamma, apply_beta)
    nc = _CACHE[key_prog]

    in_maps = []
    for k in range(N_CORES):
        m = {
            "m": np.ascontiguousarray(M_host[k].reshape(NBLK, P, TTmax * F)),
            "enc": enc[k],
            "wt": wt,
            "iota": iota_b,
        }
        if apply_bias:
            m["bias"] = np.ascontiguousarray(np.tile(b, (P, 1)))
        if apply_gamma:
            m["gamma"] = np.ascontiguousarray(np.tile(gamma, (P, 1)))
        if apply_beta:
            m["beta"] = np.ascontiguousarray(np.tile(beta, (P, 1)))
        in_maps.append(m)

    res = bass_utils.run_bass_kernel_spmd(nc, in_maps, list(range(N_CORES)))

    # ---- unshard: inverse row permutation ----
    out = np.empty((L, G, D), dtype=np.float32)
    ridx = np.arange(G, dtype=np.int64)
    dst = slotmap[block_of_row] * P + localrow
    for k in range(N_CORES):
        sel = coremap[block_of_row[ridx]] == k
        out[:, ridx[sel], :] = res.results[k]["out"][dst[sel], :, :].transpose(1, 0, 2)
    return out


# revision 23
# speedup vs baseline: 1.0502x; 1.0502x over previous
"""Trainium2 Bass kernel for batched GNN message passing.

Computes, for x:[L,G,D], COO edges (rows, cols, vals), W:[D,D], b, gamma, beta:
    xt  = x.transpose(1,0,2).reshape(G, L*D)
    agg = segment_sum(xt[cols] * vals[:,None], rows, G)     # [G, L*D]
    h   = einsum('lgd,od->lgo', agg_as_lgd, W) + b
    s   = silu(h)
    out = layernorm(s) * gamma + beta                        # LN over D

Strategy (v3): destination rows are LPT-packed on the host into 392
balanced 128-row blocks (49 per core, 16 tiles of 128 edges each).  The
host routes each edge to its destination block and lays the source
features xt[cols] out as dense bf16 message tiles M[block][p][tile][L*D]
(a pure permutation/copy), streamed to the device with large regular
HWDGE DMAs — no gpsimd dma_gather (whose per-index SWDGE descriptor
generation was the v1 bottleneck), and no gpsimd compute at all (its
SBUF port sharing with the vector engine poisons DVE throughput).

The per-tile one-hot-times-val selection matrix S[e,r] = val_e*(rowloc_e
== r) for ALL 16 tiles of a block is built in ONE custom-DVE instruction:
host packs enc[p,t] = rowloc + val (val in (0,1); val==0 edges dropped),
and the op computes t = enc - iota_r; S = relu(t)*(t <= 1), which equals
val exactly at r == rowloc and 0 elsewhere.  The segment-sum runs
directly in transposed form on the PE: aggT[d,r] += M_l[e,d].T @ S[e,r],
so the 128x128 linear consumes aggT with no on-chip transpose.  SiLU and
the squared-sum for LayerNorm run on ACT with stream accumulators.
"""

import numpy as np

L, G, D, E = 2, 50000, 128, 800000
N_CORES = 8
P = 128
NBLK = 49                     # block slots per core
NBLK_TOT = N_CORES * NBLK     # 392 blocks of 128 rows = 50176 slots
RPC = NBLK * P                # padded rows per core = 6272
F = L * D                     # 256 packed feature width
LN_EPS = 1e-5

_CACHE: dict = {}
_GNN_SEL = None


def _register_dve_op():
    """Register (once) the custom DVE op building val*onehot(rowloc) tiles.

    out[p, s, k] = relu(t) * (t <= 1),  t = in1[p, s, 0] - in0[p, 0, k]
    With in0 = iota (k) and in1 = rowloc + val (val in (0,1]):
      k == rowloc -> t = val  -> out = val
      k <  rowloc -> t >= 1+val > 1 -> masked to 0 (val > 0)
      k >  rowloc -> t <= val-1 <= 0 -> relu gives 0
      padding (enc = 0) -> t = -k <= 0 -> 0
    """
    global _GNN_SEL
    if _GNN_SEL is not None:
        return _GNN_SEL
    import re

    from concourse import dve_ops
    from concourse.dve_spec import One, Spec, Src0, Src1, relu

    for op in dve_ops.OPS:
        if op.name == "GNN_ONEHOT_SEL":
            _GNN_SEL = op
            return op

    t = Src1 - Src0
    body = relu(t) * (t <= One)
    spec = Spec(
        body=body,
        reference=lambda in0, in1, *a: np.where(
            (in1 - in0 > 0) & (in1 - in0 <= 1), in1 - in0, 0.0
        ).astype(np.float32),
    )
    op = dve_ops.DveOp("GNN_ONEHOT_SEL", spec, subdim=False, uops_sha={})
    dve_ops.OPS.append(op)
    row = dve_ops._CUSTOM_DVE_ROW_BASE + len(dve_ops.OPS) - 1
    assert row < 0x20, "custom-DVE row field overflow"
    dve_ops._SUB_OPCODE_FOR_NAME[op.name] = row
    dve_ops.CUSTOM_DVE_SPECS[op.name] = spec
    for ver in ("v3", "v4"):
        try:
            op.compile(ver)
        except ValueError as e:
            m = re.search(r'uops_sha\["%s"\]="([0-9a-f]+)"' % ver, str(e))
            if m:
                op.uops_sha[ver] = m.group(1)
        try:
            op.compile(ver)
        except ValueError:
            pass
    _GNN_SEL = op
    return op


def _build_program(TT, apply_bias, apply_gamma, apply_beta):
    import concourse.bacc as bacc
    import concourse.mybir as mybir
    import concourse.tile as tile

    sel_op = _register_dve_op()

    f32 = mybir.dt.float32
    bf16 = mybir.dt.bfloat16
    Alu = mybir.AluOpType
    Act = mybir.ActivationFunctionType

    TTmax = max(TT)
    NCOL = NBLK * L

    nc = bacc.Bacc(None, target_bir_lowering=False, debug=False)

    m_d = nc.dram_tensor("m", [NBLK, P, TTmax * F], bf16, kind="ExternalInput")
    enc_d = nc.dram_tensor("enc", [P, NBLK * TTmax], f32, kind="ExternalInput")
    wt_d = nc.dram_tensor("wt", [P, P], bf16, kind="ExternalInput")
    iota_d = nc.dram_tensor("iota", [P, P], f32, kind="ExternalInput")
    if apply_bias:
        bias_d = nc.dram_tensor("bias", [P, P], f32, kind="ExternalInput")
    if apply_gamma:
        gamma_d = nc.dram_tensor("gamma", [P, P], f32, kind="ExternalInput")
    if apply_beta:
        beta_d = nc.dram_tensor("beta", [P, P], f32, kind="ExternalInput")
    out_d = nc.dram_tensor("out", [RPC, L, D], f32, kind="ExternalOutput")

    with tile.TileContext(nc) as tc:
        with (
            tc.tile_pool(name="const", bufs=1) as constp,
            tc.tile_pool(name="mbuf", bufs=4) as mpool,
            tc.tile_pool(name="sbuild", bufs=3) as spool,
            tc.tile_pool(name="mid", bufs=4) as midpool,
            tc.tile_pool(name="store", bufs=1) as store,
            tc.tile_pool(name="outp", bufs=3) as outp,
            tc.tile_pool(name="psA", bufs=2, space="PSUM") as psA,
            tc.tile_pool(name="psB", bufs=2, space="PSUM") as psB,
            tc.tile_pool(name="psH", bufs=2, space="PSUM") as psH,
        ):
            wt_s = constp.tile([P, P], bf16)
            nc.sync.dma_start(wt_s[:], wt_d[:])
            iota_s = constp.tile([P, P], f32)
            nc.sync.dma_start(iota_s[:], iota_d[:])
            enc_s = constp.tile([P, NBLK * TTmax], f32)
            nc.sync.dma_start(enc_s[:], enc_d[:])
            if apply_bias:
                bias_s = constp.tile([P, P], f32)
                nc.sync.dma_start(bias_s[:], bias_d[:])
            if apply_gamma:
                gamma_s = constp.tile([P, P], f32)
                nc.sync.dma_start(gamma_s[:], gamma_d[:])
            if apply_beta:
                beta_s = constp.tile([P, P], f32)
                nc.sync.dma_start(beta_s[:], beta_d[:])

            s_store = store.tile([P, NCOL * P], bf16)
            muvar = store.tile([P, NCOL, 2], f32)
            eps_t = store.tile([P, 1], f32)
            nc.vector.memset(eps_t[:], LN_EPS)

            # ---- Phase 1: stream M, build S, segment-sum, linear, SiLU ----
            for bi in range(NBLK):
                tt = TT[bi]

                M = mpool.tile([P, TTmax, F], bf16, tag="m")
                nc.sync.dma_start(M[:, :tt, :], m_d[bi][:, : tt * F])

                S_all = spool.tile([P, TTmax, P], bf16, tag="s")
                nc.vector._custom_dve(
                    sel_op,
                    out=S_all[:, :tt, :],
                    in0=iota_s[:].unsqueeze(1).broadcast_to([P, tt, P]),
                    in1=enc_s[:, bi * TTmax : bi * TTmax + tt]
                    .unsqueeze(2)
                    .broadcast_to([P, tt, P]),
                )

                # two accumulation chains in separate PSUM banks (interleaved
                # groups within one bank corrupt each other — HW-probed)
                agg0 = psA.tile([P, P], f32, tag="a0")
                agg1 = psB.tile([P, P], f32, tag="a1")
                for t in range(tt):
                    nc.tensor.matmul(
                        agg0[:], lhsT=M[:, t, 0:P], rhs=S_all[:, t, :],
                        start=(t == 0), stop=(t == tt - 1),
                    )
                for t in range(tt):
                    nc.tensor.matmul(
                        agg1[:], lhsT=M[:, t, P:F], rhs=S_all[:, t, :],
                        start=(t == 0), stop=(t == tt - 1),
                    )

                aT = midpool.tile([P, L, P], bf16, tag="aT")
                nc.scalar.activation(out=aT[:, 0, :], in_=agg0[:], func=Act.Copy)
                nc.scalar.activation(out=aT[:, 1, :], in_=agg1[:], func=Act.Copy)
                for l in range(L):
                    col = bi * L + l
                    h_ps = psH.tile([P, P], f32, tag="h")
                    nc.tensor.matmul(
                        h_ps[:], lhsT=aT[:, l, :], rhs=wt_s[:], start=True, stop=True
                    )
                    if apply_bias:
                        hb = outp.tile([P, P], f32, tag="hb")
                        nc.vector.tensor_tensor(
                            out=hb[:], in0=h_ps[:], in1=bias_s[:], op=Alu.add
                        )
                        silu_in = hb[:]
                    else:
                        silu_in = h_ps[:]
                    s_sl = s_store[:, col * P : (col + 1) * P]
                    nc.scalar.activation(out=s_sl, in_=silu_in, func=Act.Silu)
                    bn6 = outp.tile([P, 6], f32, tag="bn6")
                    nc.vector.bn_stats(bn6[:], s_sl)
                    nc.vector.bn_aggr(muvar[:, col, :], bn6[:])

                # fused LayerNorm + store for this block
                std2 = outp.tile([P, L], f32, tag="std2")
                nc.scalar.activation(
                    out=std2[:],
                    in_=muvar[:, bi * L : (bi + 1) * L, 1],
                    func=Act.Sqrt,
                    bias=eps_t[:],
                )
                rstd2 = outp.tile([P, L], f32, tag="rstd2")
                nc.vector.reciprocal(rstd2[:], std2[:])
                nmr2 = outp.tile([P, L], f32, tag="nmr2")
                nc.vector.tensor_tensor(
                    out=nmr2[:], in0=muvar[:, bi * L : (bi + 1) * L, 0],
                    in1=rstd2[:], op=Alu.mult,
                )
                nc.vector.tensor_scalar(
                    out=nmr2[:], in0=nmr2[:], scalar1=-1.0, scalar2=None,
                    op0=Alu.mult,
                )
                o_t = outp.tile([P, L, P], f32, tag="o")
                for l in range(L):
                    col = bi * L + l
                    nc.scalar.activation(
                        out=o_t[:, l, :],
                        in_=s_store[:, col * P : (col + 1) * P],
                        func=Act.Identity,
                        scale=rstd2[:, l : l + 1],
                        bias=nmr2[:, l : l + 1],
                    )
                    if apply_gamma:
                        nc.vector.tensor_tensor(
                            out=o_t[:, l, :], in0=o_t[:, l, :], in1=gamma_s[:],
                            op=Alu.mult,
                        )
                    if apply_beta:
                        nc.vector.tensor_tensor(
                            out=o_t[:, l, :], in0=o_t[:, l, :], in1=beta_s[:],
                            op=Alu.add,
                        )
                nc.sync.dma_start(out_d[bi * P : (bi + 1) * P], o_t[:])

    nc.compile()
    return nc


def _pack_rows(deg):
    """LPT-pack G rows into NBLK_TOT blocks of exactly P rows, balancing
    total edge load.  Returns (block_of_row, localrow_of_row, load)."""
    import heapq

    order = np.argsort(-deg, kind="stable")
    heap = [(0, 0, b) for b in range(NBLK_TOT)]  # (load, nrows, block)
    heapq.heapify(heap)
    block_of_row = np.empty(G, dtype=np.int64)
    localrow = np.empty(G, dtype=np.int64)
    load_arr = np.zeros(NBLK_TOT, dtype=np.int64)
    for r in order:
        while True:
            load, cnt, b = heapq.heappop(heap)
            if cnt < P:
                break
        block_of_row[r] = b
        localrow[r] = cnt
        load_arr[b] = load + deg[r]
        heapq.heappush(heap, (load + int(deg[r]), cnt + 1, b))
    return block_of_row, localrow, load_arr


def kernel(x, rows, cols, vals, W, b, gamma, beta):
    import ml_dtypes
    from concourse import bass_utils

    x = np.asarray(x, dtype=np.float32)
    rows = np.asarray(rows, dtype=np.int64)
    cols = np.asarray(cols, dtype=np.int64)
    vals = np.asarray(vals, dtype=np.float32)
    W = np.asarray(W, dtype=np.float32)
    b = np.asarray(b, dtype=np.float32)
    gamma = np.asarray(gamma, dtype=np.float32)
    beta = np.asarray(beta, dtype=np.float32)
    bf = ml_dtypes.bfloat16

    # zero-valued edges contribute nothing; drop them (required by the
    # enc = rowloc + val encoding, which needs val > 0)
    keep = vals != 0.0
    if not keep.all():
        rows, cols, vals = rows[keep], cols[keep], vals[keep]
    ne = len(rows)

    # ---- host-side routing: balanced destination blocks ----
    deg = np.bincount(rows, minlength=G)
    block_of_row, localrow, load = _pack_rows(deg)

    rank = np.argsort(-load, kind="stable")
    coremap = np.empty(NBLK_TOT, dtype=np.int64)
    slotmap = np.empty(NBLK_TOT, dtype=np.int64)
    for i in range(NBLK_TOT):
        coremap[rank[i]] = i % N_CORES
        slotmap[rank[i]] = i // N_CORES
    slot_load = np.zeros(NBLK, dtype=np.int64)
    for bk in range(NBLK_TOT):
        slot_load[slotmap[bk]] = max(slot_load[slotmap[bk]], load[bk])
    TT = [max(1, int(v)) for v in np.ceil(slot_load / P).astype(np.int64)]
    TTmax = max(TT)

    # ---- route edges ----
    eb = block_of_row[rows]
    core_e = coremap[eb]
    slot_e = slotmap[eb]
    rowloc_e = localrow[rows].astype(np.float32)
    gid = core_e * NBLK + slot_e
    order = np.argsort(gid, kind="stable")
    gid_s = gid[order]
    counts = np.bincount(gid_s, minlength=N_CORES * NBLK)
    starts = np.zeros(N_CORES * NBLK, dtype=np.int64)
    np.cumsum(counts[:-1], out=starts[1:])
    pos = np.arange(ne, dtype=np.int64) - starts[gid_s]
    t_arr = pos // P
    p_arr = pos % P
    core_s = core_e[order]
    slot_s = slot_e[order]

    # ---- message tiles: pure gather/permutation of xt, in bf16 ----
    xt = np.ascontiguousarray(x.transpose(1, 0, 2).reshape(G, F)).astype(bf)
    M_host = np.zeros((N_CORES, NBLK, P, TTmax, F), dtype=bf)
    M_host[core_s, slot_s, p_arr, t_arr] = xt[cols[order]]

    # enc[p, slot*TTmax + t] = rowloc + val   (0 in padding slots).
    # If val is so small that rowloc+val rounds to exactly rowloc, the
    # device decode would read it as a full-weight edge into rowloc-1;
    # zero it instead (its true contribution is < 8e-6).
    encv = rowloc_e[order] + vals[order]
    encv[encv == rowloc_e[order]] = 0.0
    enc = np.zeros((N_CORES, P, NBLK * TTmax), dtype=np.float32)
    enc[core_s, p_arr, slot_s * TTmax + t_arr] = encv

    wt = np.ascontiguousarray(W.T).astype(bf)
    iota_b = np.ascontiguousarray(
        np.tile(np.arange(P, dtype=np.float32), (P, 1))
    )

    apply_bias = bool(np.any(b != 0))
    apply_gamma = bool(np.any(gamma != 1))
    apply_beta = bool(np.any(beta != 0))

    key_prog = (tuple(TT), apply_bias, apply_gamma, apply_beta)
    if key_prog not in _CACHE:
        _CACHE[key_prog] = _build_program(TT, apply_bias, apply_gamma, apply_beta)
    nc = _CACHE[key_prog]

    in_maps = []
    for k in range(N_CORES):
        m = {
            "m": np.ascontiguousarray(M_host[k].reshape(NBLK, P, TTmax * F)),
            "enc": enc[k],
            "wt": wt,
            "iota": iota_b,
        }
        if apply_bias:
            m["bias"] = np.ascontiguousarray(np.tile(b, (P, 1)))
        if apply_gamma:
            m["gamma"] = np.ascontiguousarray(np.tile(gamma, (P, 1)))
        if apply_beta:
            m["beta"] = np.ascontiguousarray(np.tile(beta, (P, 1)))
        in_maps.append(m)

    res = bass_utils.run_bass_kernel_spmd(nc, in_maps, list(range(N_CORES)))

    # ---- unshard: inverse row permutation ----
    out = np.empty((L, G, D), dtype=np.float32)
    ridx = np.arange(G, dtype=np.int64)
    dst = slotmap[block_of_row] * P + localrow
    for k in range(N_CORES):
        sel = coremap[block_of_row[ridx]] == k
        out[:, ridx[sel], :] = res.results[k]["out"][dst[sel], :, :].transpose(1, 0, 2)
    return out
